# revision 7
# baseline (speedup 1.0000x reference)
"""Distributed multi-head attention kernel for 8 TRN2 NeuronCores.

Problem: B=2, S=2048, H=1024 (16 heads x 64), fp32 in/out.
Sharding: core c = 4*b + g handles batch b and head-group g (4 heads, 256
hidden cols). Wq/Wk/Wv column-sharded, Wo row-sharded; a per-q-chunk
bf16 ReduceScatter over each 4-core batch group yields each core's
4x128-row slices of the output (overlapped with compute).

v2 dataflow per core (transpose-free attention, bf16 matmuls, fp32 PSUM):
  x^T via DMA-XBAR transpose split across the two HWDGE queues (sync +
  scalar) so descriptor issue is 2x parallel; K/Q projections as N=1024
  matmuls chase the transposes chunk by chunk.
  V projection (x^T-stationary, N=256) is interleaved into chunk 0's
  attention so it hides under the exp/score pipeline.
  scores^T[k,q] = K^T.T@Q^T, two heads packed into PE row groups;
  Pt = exp(scores/8) with no max subtraction (scores ~ N(0,8^2), exact
  softmax); most tiles on ACT, a fraction on DVE via the exp2 bit trick
  (y = round(scores*c1 + c2) as int16 IS the bf16 encoding of
  2^(scores*log2e/8) with piecewise-linear mantissa, ~2% rms weight
  error that cancels in the softmax normalization).
  ctx^T[d,q] (+ sums row via ones column in V) = [V|1].T @ Pt.
  The per-chunk epilogue (normalize via K=1 broadcast matmul of 1/sums,
  out partial = ctx^T-stationary @ Wo, bf16 ReduceScatter) is deferred
  and interleaved into the NEXT chunk's attention so the in-order PE/ACT
  queues never drain between chunks.
bq/bk applied on-device (ACT bias); bv/bo folded in on host (exact:
out += bv@Wo + bo, since softmax rows sum to one).
"""

import sys

for p in ("/opt/trn_rl_repo",):
    if p not in sys.path:
        sys.path.insert(0, p)

import math
from contextlib import ExitStack

import ml_dtypes
import numpy as np

from concourse import bacc, mybir, tile
from concourse.bass import ds
from concourse.bass_utils import run_bass_kernel_spmd

F32 = mybir.dt.float32
BF16 = mybir.dt.bfloat16
I16 = mybir.dt.int16
AF = mybir.ActivationFunctionType

B, S, H = 2, 2048, 1024
NH, D = 16, 64
NCORES = 8
GROUPS = [[0, 1, 2, 3], [4, 5, 6, 7]]
JG = 256           # hidden cols per core (4 heads)
SO = S // 4        # 512 output rows per core after reduce-scatter

# exp2 bit-trick constants: bits_i16 = scores * C1 + C2 viewed as bf16
# encodes 2^(scores/8 * log2 e) with a piecewise-linear mantissa.
EXP_C1 = 128.0 * 0.125 * math.log2(math.e)
EXP_C2 = 128.0 * (127.0 - 0.0579)
# kt indices whose exp tile runs on DVE (per hp pass) in chunks 1..3
DVE_KT = (2, 5, 8, 11, 14)

_cache = {}


def _build():
    nc = bacc.Bacc("TRN2", target_bir_lowering=False, debug=False,
                   num_devices=NCORES)
    x_d = nc.dram_tensor("xbf", [S, H], BF16, kind="ExternalInput")
    wq_d = nc.dram_tensor("wq", [H, JG], BF16, kind="ExternalInput")
    wk_d = nc.dram_tensor("wk", [H, JG], BF16, kind="ExternalInput")
    wv_d = nc.dram_tensor("wv", [H, JG], BF16, kind="ExternalInput")
    wo_d = nc.dram_tensor("wo", [JG, H], BF16, kind="ExternalInput")
    bq_d = nc.dram_tensor("bqc", [128, 2], F32, kind="ExternalInput")
    bk_d = nc.dram_tensor("bkc", [128, 2], F32, kind="ExternalInput")
    out_d = nc.dram_tensor("out", [SO, H], BF16, kind="ExternalOutput")

    def mm(ps, lhsT, rhs, start, stop, tile_position=None):
        nc.tensor.matmul(ps, lhsT, rhs, start=start, stop=stop,
                         tile_position=tile_position)

    with tile.TileContext(nc) as tc, ExitStack() as st:
        consts = st.enter_context(tc.tile_pool(name="consts", bufs=1))
        ones1 = consts.tile([1, 64], BF16)
        nc.vector.memset(ones1[:], 1.0)
        bq_sb = consts.tile([128, 2], F32)
        nc.gpsimd.dma_start(bq_sb[:], bq_d[:, :])
        bk_sb = consts.tile([128, 2], F32)
        nc.gpsimd.dma_start(bk_sb[:], bk_d[:, :])

        wpool = st.enter_context(tc.tile_pool(name="weights", bufs=1))
        w_sb = {}

        def load_w(wname, wd, inner):
            # w*_sb[:, inner*s + j] = W[s*128 + p, j]; on the scalar queue
            wt = wpool.tile([128, 2048], BF16, name=f"{wname}sb",
                            tag=f"{wname}sb")
            nc.scalar.dma_start(
                wt[:].rearrange("p (s j) -> p s j", j=inner),
                wd.ap().rearrange("(s p) j -> p s j", p=128))
            w_sb[wname] = wt

        qkv = st.enter_context(tc.tile_pool(name="qkv", bufs=1))
        qT = [qkv.tile([128, S], BF16, name=f"qT{j}", tag=f"qT{j}")
              for j in range(2)]
        kT = [qkv.tile([128, S], BF16, name=f"kT{j}", tag=f"kT{j}")
              for j in range(2)]
        ctxT = [qkv.tile([128, S], BF16, name=f"cT{j}", tag=f"cT{j}")
                for j in range(2)]
        # V padded per head with a ones column: head h at cols 65h..65h+63
        v_sb = [qkv.tile([128, 260], BF16, name=f"v{i}", tag=f"v{i}")
                for i in range(16)]

        xTp = st.enter_context(tc.tile_pool(name="xT", bufs=1))
        xT = [xTp.tile([128, S], BF16, name=f"xT{s}", tag=f"xT{s}")
              for s in range(8)]

        dram = st.enter_context(tc.tile_pool(name="dram", bufs=1, space="DRAM"))
        partial_c = [dram.tile([512, H], BF16, name=f"pc{i}", tag=f"pc{i}")
                     for i in range(4)]
        rs_c = [dram.tile([128, H], BF16, name=f"rc{i}", tag=f"rc{i}")
                for i in range(4)]

        scps = st.enter_context(tc.tile_pool(name="scps", bufs=2, space="PSUM"))
        ctxps = st.enter_context(tc.tile_pool(name="ctxps", bufs=3,
                                              space="PSUM"))
        psb = st.enter_context(tc.tile_pool(name="psb", bufs=4))
        rsb = st.enter_context(tc.tile_pool(name="rsb", bufs=2))
        osb = st.enter_context(tc.tile_pool(name="osb", bufs=4))

        # ---- Phase A: x^T via dual-queue DMA XBAR transpose + K/Q proj ----
        load_w("wk", wk_d, 256)
        load_w("wq", wq_d, 256)

        def transposes(tq):
            # all on the sync queue: concurrent XBAR transposes from the
            # two HWDGE queues corrupt each other (shared S2M XBAR state)
            for s in range(8):
                nc.sync.dma_start(xT[s][:, ds(512 * tq, 512)],
                                  x_d.ap()[ds(512 * tq, 512), ds(128 * s, 128)],
                                  transpose=True)

        def proj_pair(wt, dst, jt, tqp):
            # dst[jt][:, 1024*tqp:+1024] = (W^T x^T) for a 2-chunk slice
            # (two N=512 matmul groups: ISA caps matmul output at 512 elems)
            ps = scps.tile([128, 1024], F32, tag="sps")
            for half in range(2):
                for s in range(8):
                    mm(ps[:, ds(512 * half, 512)],
                       wt[:, ds(256 * s + 128 * jt, 128)],
                       xT[s][:, ds(1024 * tqp + 512 * half, 512)],
                       s == 0, s == 7)
            return ps

        transposes(0)
        transposes(1)
        qk_ps = []
        for jt in range(2):
            qk_ps.append(("k", jt, 0, proj_pair(w_sb["wk"], kT, jt, 0)))
        transposes(2)
        load_w("wv", wv_d, 256)
        transposes(3)
        load_w("wo", wo_d, 1024)
        for jt in range(2):
            qk_ps.append(("k", jt, 1, proj_pair(w_sb["wk"], kT, jt, 1)))
        for jt in range(2):
            qk_ps.append(("q", jt, 0, proj_pair(w_sb["wq"], qT, jt, 0)))
        for kind, jt, tqp, ps in qk_ps:
            dst, bias = (kT, bk_sb) if kind == "k" else (qT, bq_sb)
            nc.scalar.activation(dst[jt][:, ds(1024 * tqp, 1024)], ps[:],
                                 AF.Identity, bias=bias[:, ds(jt, 1)])
        del qk_ps

        # scoped PSUM bank: V-projection accumulator during chunk 0, then
        # reused as the broadcast bank for the deferred epilogues.
        vps_cm = tc.tile_pool(name="vps", bufs=1, space="PSUM")
        vps = vps_cm.__enter__()
        bcps = None

        def v_proj(tv):
            ps = vps.tile([128, 256], F32, tag="vps")
            for s in range(8):
                mm(ps[:], xT[s][:, ds(128 * tv, 128)],
                   w_sb["wv"][:, ds(256 * s, 256)], s == 0, s == 7)
            nc.vector.memset(v_sb[tv][:], 1.0)
            nc.vector.tensor_copy(
                v_sb[tv][:].rearrange("p (h c) -> p h c", c=65)[:, :, 0:64],
                ps[:].rearrange("p (h c) -> p h c", c=64))

        def q_proj_half(tq, jt):
            ps = scps.tile([128, 1024], F32, tag="sps")
            for s in range(8):
                mm(ps[:, 0:512], w_sb["wq"][:, ds(256 * s + 128 * jt, 128)],
                   xT[s][:, ds(512 * tq, 512)], s == 0, s == 7)
            nc.scalar.activation(qT[jt][:, ds(512 * tq, 512)], ps[:, 0:512],
                                 AF.Identity, bias=bq_sb[:, ds(jt, 1)])

        # ---- Chunks: attention with deferred, interleaved epilogue ----
        # pending[tq] = list of 4 (h, cu, r16) in normalize order
        pending = {}

        def epi_norm(item):
            # broadcast 1/sums over 64 partitions and scale ctx -> ctxT
            h, cu, r16, tq = item
            hp, po = h // 2, 64 * (h % 2)
            bc = bcps.tile([64, 512], F32, tag="bc")
            mm(bc[:], ones1[:], r16[:], True, True)
            nc.vector.tensor_mul(
                ctxT[hp][ds(po, 64), ds(512 * tq, 512)], cu[0:64, :], bc[:])

        def epi_outproj(tq, tt):
            ps = scps.tile([128, 1024], F32, tag="sps")
            for oo in range(2):
                for idx, js in enumerate((1, 0)):
                    mm(ps[:, ds(512 * oo, 512)],
                       ctxT[js][:, ds(512 * tq + 128 * tt, 128)],
                       w_sb["wo"][:, ds(1024 * js + 512 * oo, 512)],
                       idx == 0, idx == 1)
            ot = osb.tile([128, 1024], BF16, tag="ot")
            nc.vector.tensor_copy(ot[:], ps[:])
            nc.sync.dma_start(partial_c[tq][ds(128 * tt, 128), :], ot[:])

        def epi_rs(tq):
            nc.gpsimd.collective_compute(
                "ReduceScatter", mybir.AluOpType.add,
                replica_groups=GROUPS,
                ins=[partial_c[tq].opt()], outs=[rs_c[tq].opt()])
            nc.sync.dma_start(out_d[ds(128 * tq, 128), :], rs_c[tq][:])

        for tq in range(4):
            prev = pending.pop(tq - 1, None)
            cus = []
            for hp in (1, 0):
                cA = ctxps.tile([65, 512], F32, tag="cps")
                cB = ctxps.tile([65, 512], F32, tag="cps")
                for kt in range(16):
                    # chunk 0: V projection rides along the first hp pass
                    if tq == 0 and hp == 1:
                        v_proj(kt)
                    # deferred epilogue of the previous chunk
                    if prev is not None and hp == 1:
                        if 2 <= kt <= 5:
                            epi_norm(prev[kt - 2])
                        elif kt in (7, 9, 11, 13):
                            epi_outproj(tq - 1, (kt - 7) // 2)
                        elif kt == 15:
                            epi_rs(tq - 1)
                    # Q projection for chunk tq+1 (chunks 1,2 carry Q2,Q3)
                    if tq in (1, 2) and hp == 0 and kt in (2, 6):
                        q_proj_half(tq + 1, kt // 4)
                    sp = scps.tile([128, 1024], F32, tag="sps")
                    mm(sp[:, 0:512],
                       kT[hp][0:64, ds(128 * kt, 128)],
                       qT[hp][0:64, ds(512 * tq, 512)],
                       True, True, tile_position=(0, 0))
                    mm(sp[:, 512:1024],
                       kT[hp][64:128, ds(128 * kt, 128)],
                       qT[hp][64:128, ds(512 * tq, 512)],
                       True, True, tile_position=(64, 0))
                    pt = psb.tile([128, 1024], BF16, tag="pt")
                    if tq > 0 and kt in DVE_KT:
                        nc.vector.tensor_scalar(
                            pt[:].bitcast(I16), sp[:], EXP_C1, EXP_C2,
                            mybir.AluOpType.mult, mybir.AluOpType.add)
                    else:
                        nc.scalar.activation(pt[:], sp[:], AF.Exp, scale=0.125)
                    mm(cA[:], v_sb[kt][:, ds(65 * (2 * hp), 65)],
                       pt[:, 0:512], kt == 0, kt == 15)
                    mm(cB[:], v_sb[kt][:, ds(65 * (2 * hp + 1), 65)],
                       pt[:, 512:1024], kt == 0, kt == 15)
                if tq == 0 and hp == 1:
                    # V accumulator bank becomes the broadcast bank
                    vps_cm.__exit__(None, None, None)
                    bcps_cm = tc.tile_pool(name="bcps", bufs=1, space="PSUM")
                    bcps = bcps_cm.__enter__()
                    st.push(bcps_cm)
                for h, cps in ((2 * hp, cA), (2 * hp + 1, cB)):
                    # evacuate PSUM right away; reciprocal on the thin sums
                    # row, cast bf16 for the K=1 broadcast matmul
                    cu = rsb.tile([65, 512], F32, tag="cu", bufs=8)
                    nc.vector.tensor_copy(cu[:], cps[:])
                    # custom-DVE ops misread nonzero base partitions on HW:
                    # stage the sums row at partition 0 before reciprocal
                    sm32 = rsb.tile([1, 512], F32, tag="sm32", bufs=8)
                    nc.vector.tensor_copy(sm32[:], cu[ds(64, 1), :])
                    r32 = rsb.tile([1, 512], F32, tag="r32", bufs=8)
                    nc.vector.reciprocal_approx_fast(r32[:], sm32[:])
                    r16 = rsb.tile([1, 512], BF16, tag="r16", bufs=8)
                    nc.vector.tensor_copy(r16[:], r32[:])
                    cus.append((h, cu, r16, tq))
            pending[tq] = cus

        # tail: chunk 3 epilogue runs exposed
        last = pending.pop(3)
        for item in last:
            epi_norm(item)
        for tt in range(4):
            epi_outproj(3, tt)
        epi_rs(3)

    nc.compile()
    return nc


def _get_nc():
    if "nc" not in _cache:
        _cache["nc"] = _build()
    return _cache["nc"]


def _in_maps(x, Wq, bq, Wk, bk, Wv, bv, Wo, bo):
    bf = ml_dtypes.bfloat16
    maps = []
    for c in range(NCORES):
        b, g = c // 4, c % 4
        j0 = JG * g
        maps.append({
            "xbf": np.ascontiguousarray(x[b]).astype(bf),
            "wq": np.ascontiguousarray(Wq[:, j0:j0 + JG]).astype(bf),
            "wk": np.ascontiguousarray(Wk[:, j0:j0 + JG]).astype(bf),
            "wv": np.ascontiguousarray(Wv[:, j0:j0 + JG]).astype(bf),
            "wo": np.ascontiguousarray(Wo[j0:j0 + JG, :]).astype(bf),
            "bqc": np.ascontiguousarray(bq[j0:j0 + JG].reshape(2, 128).T),
            "bkc": np.ascontiguousarray(bk[j0:j0 + JG].reshape(2, 128).T),
        })
    return maps


def kernel(x, Wq, bq, Wk, bk, Wv, bv, Wo, bo, _trace=False):
    x, Wq, bq, Wk, bk, Wv, bv, Wo, bo = (
        np.asarray(a, dtype=np.float32)
        for a in (x, Wq, bq, Wk, bk, Wv, bv, Wo, bo))
    nc = _get_nc()
    res = run_bass_kernel_spmd(nc, _in_maps(x, Wq, bq, Wk, bk, Wv, bv, Wo, bo),
                               core_ids=list(range(NCORES)), trace=_trace)
    out = np.empty((B, S, H), np.float32)
    for c in range(NCORES):
        b, g = c // 4, c % 4
        oc = np.asarray(res.results[c]["out"], dtype=np.float32)
        for tq in range(4):
            out[b, 512 * tq + 128 * g:512 * tq + 128 * (g + 1), :] = \
                oc[128 * tq:128 * (tq + 1)]
    out += bv @ Wo + bo  # exact: softmax rows sum to 1
    if _trace:
        return out, res
    return out


# revision 17
# speedup vs baseline: 1.2177x; 1.2177x over previous
"""Distributed multi-head attention kernel for 8 TRN2 NeuronCores.

Problem: B=2, S=2048, H=1024 (16 heads x 64), fp32 in/out.
Sharding: core c = 4*b + g handles batch b and head-group g (4 heads, 256
hidden cols). Wq/Wk/Wv column-sharded, Wo row-sharded; a per-q-chunk
bf16 ReduceScatter over each 4-core batch group yields each core's
4x128-row slices of the output (overlapped with compute).

v2 dataflow per core (transpose-free attention, bf16 matmuls, fp32 PSUM):
  x^T via DMA-XBAR transpose split across the two HWDGE queues (sync +
  scalar) so descriptor issue is 2x parallel; K/Q projections as N=1024
  matmuls chase the transposes chunk by chunk.
  V projection (x^T-stationary, N=256) is interleaved into chunk 0's
  attention so it hides under the exp/score pipeline.
  scores^T[k,q] = K^T.T@Q^T, two heads packed into PE row groups;
  Pt = exp(scores/8) with no max subtraction (scores ~ N(0,8^2), exact
  softmax); most tiles on ACT, a fraction on DVE via the exp2 bit trick
  (y = round(scores*c1 + c2) as int16 IS the bf16 encoding of
  2^(scores*log2e/8) with piecewise-linear mantissa, ~2% rms weight
  error that cancels in the softmax normalization).
  ctx^T[d,q] (+ sums row via ones column in V) = [V|1].T @ Pt.
  The per-chunk epilogue (normalize via K=1 broadcast matmul of 1/sums,
  out partial = ctx^T-stationary @ Wo, bf16 ReduceScatter) is deferred
  and interleaved into the NEXT chunk's attention so the in-order PE/ACT
  queues never drain between chunks.
bq/bk applied on-device (ACT bias); bv/bo folded in on host (exact:
out += bv@Wo + bo, since softmax rows sum to one).
"""

import sys

for p in ("/opt/trn_rl_repo",):
    if p not in sys.path:
        sys.path.insert(0, p)

import math
from contextlib import ExitStack

import ml_dtypes
import numpy as np

from concourse import bacc, mybir, tile
from concourse.bass import ds
from concourse.bass_utils import run_bass_kernel_spmd

F32 = mybir.dt.float32
BF16 = mybir.dt.bfloat16
I16 = mybir.dt.int16
AF = mybir.ActivationFunctionType

B, S, H = 2, 2048, 1024
NH, D = 16, 64
NCORES = 8
GROUPS = [[0, 1, 2, 3], [4, 5, 6, 7]]
JG = 256           # hidden cols per core (4 heads)
SO = S // 4        # 512 output rows per core after reduce-scatter

# exp2 bit-trick constants: bits_i16 = scores * C1 + C2 viewed as bf16
# encodes 2^(scores/8 * log2 e) with a piecewise-linear mantissa.
EXP_C1 = 128.0 * 0.125 * math.log2(math.e)
EXP_C2 = 128.0 * (127.0 - 0.0579)
# kt indices whose exp tile runs on DVE (per hp pass) in chunks 1..3
DVE_KT = (5, 11)

_cache = {}


def _build():
    nc = bacc.Bacc("TRN2", target_bir_lowering=False, debug=False,
                   num_devices=NCORES)
    x_d = nc.dram_tensor("xbf", [S, H], BF16, kind="ExternalInput")
    id_d = nc.dram_tensor("ident", [128, 128], BF16, kind="ExternalInput")
    wq_d = nc.dram_tensor("wq", [H, JG], BF16, kind="ExternalInput")
    wk_d = nc.dram_tensor("wk", [H, JG], BF16, kind="ExternalInput")
    wv_d = nc.dram_tensor("wv", [H, JG], BF16, kind="ExternalInput")
    wo_d = nc.dram_tensor("wo", [JG, H], BF16, kind="ExternalInput")
    bq_d = nc.dram_tensor("bqc", [128, 2], F32, kind="ExternalInput")
    bk_d = nc.dram_tensor("bkc", [128, 2], F32, kind="ExternalInput")
    out_d = nc.dram_tensor("out", [SO, H], BF16, kind="ExternalOutput")

    def mm(ps, lhsT, rhs, start, stop, tile_position=None):
        nc.tensor.matmul(ps, lhsT, rhs, start=start, stop=stop,
                         tile_position=tile_position)

    with tile.TileContext(nc) as tc, ExitStack() as st:
        consts = st.enter_context(tc.tile_pool(name="consts", bufs=1))
        ones1 = consts.tile([1, 64], BF16)
        nc.vector.memset(ones1[:], 1.0)
        bq_sb = consts.tile([128, 2], F32)
        nc.gpsimd.dma_start(bq_sb[:], bq_d[:, :])
        bk_sb = consts.tile([128, 2], F32)
        nc.gpsimd.dma_start(bk_sb[:], bk_d[:, :])

        wpool = st.enter_context(tc.tile_pool(name="weights", bufs=1))
        w_sb = {}

        def load_w(wname, wd, inner):
            # w*_sb[:, inner*s + j] = W[s*128 + p, j]; on the scalar queue
            wt = wpool.tile([128, 2048], BF16, name=f"{wname}sb",
                            tag=f"{wname}sb")
            nc.scalar.dma_start(
                wt[:].rearrange("p (s j) -> p s j", j=inner),
                wd.ap().rearrange("(s p) j -> p s j", p=128))
            w_sb[wname] = wt

        qkv = st.enter_context(tc.tile_pool(name="qkv", bufs=1))
        qT = [qkv.tile([128, S], BF16, name=f"qT{j}", tag=f"qT{j}")
              for j in range(2)]
        kT = [qkv.tile([128, S], BF16, name=f"kT{j}", tag=f"kT{j}")
              for j in range(2)]
        ctxT = [qkv.tile([128, S], BF16, name=f"cT{j}", tag=f"cT{j}")
                for j in range(2)]
        # V padded per head with a ones column: head h at cols 65h..65h+63
        v_sb = [qkv.tile([128, 260], BF16, name=f"v{i}", tag=f"v{i}")
                for i in range(16)]

        xTp = st.enter_context(tc.tile_pool(name="xT", bufs=1))
        xT = [xTp.tile([128, S], BF16, name=f"xT{s}", tag=f"xT{s}")
              for s in range(8)]
        # x rows staged untransposed for the PE-transpose path (s=3..7)
        xa = [xTp.tile([128, 640], BF16, name=f"xa{t}", tag=f"xa{t}")
              for t in range(16)]

        dram = st.enter_context(tc.tile_pool(name="dram", bufs=1, space="DRAM"))
        partial_c = [dram.tile([512, H], BF16, name=f"pc{i}", tag=f"pc{i}")
                     for i in range(3)]
        rs_c = [dram.tile([128, H], BF16, name=f"rc{i}", tag=f"rc{i}")
                for i in range(3)]
        # last chunk reduce-scatters as two 256-row halves to shrink the tail
        ph_c = [dram.tile([256, H], BF16, name=f"ph{i}", tag=f"ph{i}")
                for i in range(2)]
        rh_c = [dram.tile([64, H], BF16, name=f"rh{i}", tag=f"rh{i}")
                for i in range(2)]

        scps = st.enter_context(tc.tile_pool(name="scps", bufs=2, space="PSUM"))
        ctxps = st.enter_context(tc.tile_pool(name="ctxps", bufs=3,
                                              space="PSUM"))
        psb = st.enter_context(tc.tile_pool(name="psb", bufs=4))
        rsb = st.enter_context(tc.tile_pool(name="rsb", bufs=2))
        osb = st.enter_context(tc.tile_pool(name="osb", bufs=4))

        # ---- Phase A: x^T via XBAR (s=0..2, sync queue only: concurrent
        # XBAR transposes from two HWDGE queues corrupt each other) plus a
        # PE identity-matmul transpose path (s=3..7) fed by plain DMAs ----
        ident = consts.tile([128, 128], BF16)
        nc.scalar.dma_start(ident[:], id_d.ap())
        load_w("wk", wk_d, 256)
        load_w("wq", wq_d, 256)
        for tt in range(16):
            nc.scalar.dma_start(xa[tt][:],
                                x_d.ap()[ds(128 * tt, 128), ds(384, 640)])

        def transposes(tq):
            for s in range(3):
                nc.sync.dma_start(xT[s][:, ds(512 * tq, 512)],
                                  x_d.ap()[ds(512 * tq, 512), ds(128 * s, 128)],
                                  transpose=True)

        def proj_pair(wt, dst, jt, tqp):
            # dst[jt][:, 1024*tqp:+1024] = (W^T x^T) for a 2-chunk slice
            # (two N=512 matmul groups: ISA caps matmul output at 512 elems)
            ps = scps.tile([128, 1024], F32, tag="sps")
            for half in range(2):
                for s in range(8):
                    mm(ps[:, ds(512 * half, 512)],
                       wt[:, ds(256 * s + 128 * jt, 128)],
                       xT[s][:, ds(1024 * tqp + 512 * half, 512)],
                       s == 0, s == 7)
            return ps

        for tq in range(4):
            transposes(tq)
        load_w("wv", wv_d, 256)
        load_w("wo", wo_d, 1024)

        # scoped PSUM bank: PE-transpose ping-pong during phase A, V
        # accumulator during chunk 0, broadcast bank afterwards.
        tp_cm = tc.tile_pool(name="tp", bufs=1, space="PSUM")
        tpp = tp_cm.__enter__()
        tp = tpp.tile([128, 1024], BF16, tag="tp")
        qk_ps = []
        for tq in range(4):
            for s in range(3, 8):
                half = (5 * tq + s - 3) % 2
                for q in range(4):
                    nc.tensor.transpose(
                        tp[:, ds(512 * half + 128 * q, 128)],
                        xa[4 * tq + q][:, ds(128 * (s - 3), 128)], ident[:])
                nc.vector.tensor_copy(xT[s][:, ds(512 * tq, 512)],
                                      tp[:, ds(512 * half, 512)])
            if tq == 1:
                for jt in range(2):
                    qk_ps.append(("k", jt, 0, proj_pair(w_sb["wk"], kT, jt, 0)))
            if tq == 3:
                for jt in range(2):
                    qk_ps.append(("k", jt, 1, proj_pair(w_sb["wk"], kT, jt, 1)))
                for jt in range(2):
                    qk_ps.append(("q", jt, 0, proj_pair(w_sb["wq"], qT, jt, 0)))
        for kind, jt, tqp, ps in qk_ps:
            dst, bias = (kT, bk_sb) if kind == "k" else (qT, bq_sb)
            nc.scalar.activation(dst[jt][:, ds(1024 * tqp, 1024)], ps[:],
                                 AF.Identity, bias=bias[:, ds(jt, 1)])
        del qk_ps

        tp_cm.__exit__(None, None, None)
        vps_cm = tc.tile_pool(name="vps", bufs=1, space="PSUM")
        vps = vps_cm.__enter__()
        bcps = None

        def v_proj(tv):
            ps = vps.tile([128, 256], F32, tag="vps")
            for s in range(8):
                mm(ps[:], xT[s][:, ds(128 * tv, 128)],
                   w_sb["wv"][:, ds(256 * s, 256)], s == 0, s == 7)
            nc.vector.memset(v_sb[tv][:], 1.0)
            nc.vector.tensor_copy(
                v_sb[tv][:].rearrange("p (h c) -> p h c", c=65)[:, :, 0:64],
                ps[:].rearrange("p (h c) -> p h c", c=64))

        def q_proj_half(tq, jt):
            ps = scps.tile([128, 1024], F32, tag="sps")
            for s in range(8):
                mm(ps[:, 0:512], w_sb["wq"][:, ds(256 * s + 128 * jt, 128)],
                   xT[s][:, ds(512 * tq, 512)], s == 0, s == 7)
            nc.scalar.activation(qT[jt][:, ds(512 * tq, 512)], ps[:, 0:512],
                                 AF.Identity, bias=bq_sb[:, ds(jt, 1)])

        # ---- Chunks: attention with deferred, interleaved epilogue ----
        # pending[tq] = list of 4 (h, cu, r16) in normalize order
        pending = {}

        def epi_norm(item):
            # broadcast 1/sums over 64 partitions and scale ctx -> ctxT
            h, cu, r16, tq = item
            hp, po = h // 2, 64 * (h % 2)
            bc = bcps.tile([64, 512], F32, tag="bc")
            mm(bc[:], ones1[:], r16[:], True, True)
            nc.vector.tensor_mul(
                ctxT[hp][ds(po, 64), ds(512 * tq, 512)], cu[0:64, :], bc[:])

        def epi_outproj(tq, tt, dst=None):
            ps = scps.tile([128, 1024], F32, tag="sps")
            for oo in range(2):
                for idx, js in enumerate((1, 0)):
                    mm(ps[:, ds(512 * oo, 512)],
                       ctxT[js][:, ds(512 * tq + 128 * tt, 128)],
                       w_sb["wo"][:, ds(1024 * js + 512 * oo, 512)],
                       idx == 0, idx == 1)
            ot = osb.tile([128, 1024], BF16, tag="ot")
            nc.vector.tensor_copy(ot[:], ps[:])
            if dst is None:
                dst = partial_c[tq][ds(128 * tt, 128), :]
            nc.sync.dma_start(dst, ot[:])

        def epi_rs(tq):
            nc.gpsimd.collective_compute(
                "ReduceScatter", mybir.AluOpType.add,
                replica_groups=GROUPS,
                ins=[partial_c[tq].opt()], outs=[rs_c[tq].opt()])
            nc.sync.dma_start(out_d[ds(128 * tq, 128), :], rs_c[tq][:])

        for tq in range(4):
            prev = pending.pop(tq - 1, None)
            cus = []
            for hp in (1, 0):
                cA = ctxps.tile([65, 512], F32, tag="cps")
                cB = ctxps.tile([65, 512], F32, tag="cps")
                for kt in range(16):
                    # chunk 0: V projection rides along the first hp pass
                    if tq == 0 and hp == 1:
                        v_proj(kt)
                    # deferred epilogue of the previous chunk
                    if prev is not None and hp == 1:
                        if 2 <= kt <= 5:
                            epi_norm(prev[kt - 2])
                        elif kt in (7, 9, 11, 13):
                            epi_outproj(tq - 1, (kt - 7) // 2)
                        elif kt == 15:
                            epi_rs(tq - 1)
                    # Q projection for chunk tq+1 (chunks 1,2 carry Q2,Q3)
                    if tq in (1, 2) and hp == 0 and kt in (2, 6):
                        q_proj_half(tq + 1, kt // 4)
                    sp = scps.tile([128, 1024], F32, tag="sps")
                    mm(sp[:, 0:512],
                       kT[hp][0:64, ds(128 * kt, 128)],
                       qT[hp][0:64, ds(512 * tq, 512)],
                       True, True, tile_position=(0, 0))
                    mm(sp[:, 512:1024],
                       kT[hp][64:128, ds(128 * kt, 128)],
                       qT[hp][64:128, ds(512 * tq, 512)],
                       True, True, tile_position=(64, 0))
                    pt = psb.tile([128, 1024], BF16, tag="pt")
                    if tq > 0 and kt in DVE_KT:
                        nc.vector.tensor_scalar(
                            pt[:].bitcast(I16), sp[:], EXP_C1, EXP_C2,
                            mybir.AluOpType.mult, mybir.AluOpType.add)
                    else:
                        nc.scalar.activation(pt[:], sp[:], AF.Exp, scale=0.125)
                    mm(cA[:], v_sb[kt][:, ds(65 * (2 * hp), 65)],
                       pt[:, 0:512], kt == 0, kt == 15)
                    mm(cB[:], v_sb[kt][:, ds(65 * (2 * hp + 1), 65)],
                       pt[:, 512:1024], kt == 0, kt == 15)
                if tq == 0 and hp == 1:
                    # V accumulator bank becomes the broadcast bank
                    vps_cm.__exit__(None, None, None)
                    bcps_cm = tc.tile_pool(name="bcps", bufs=1, space="PSUM")
                    bcps = bcps_cm.__enter__()
                    st.push(bcps_cm)
                for h, cps in ((2 * hp, cA), (2 * hp + 1, cB)):
                    # evacuate PSUM right away; reciprocal on the thin sums
                    # row, cast bf16 for the K=1 broadcast matmul
                    cu = rsb.tile([65, 512], F32, tag="cu", bufs=8)
                    nc.vector.tensor_copy(cu[:], cps[:])
                    # custom-DVE ops misread nonzero base partitions on HW:
                    # stage the sums row at partition 0 before reciprocal
                    sm32 = rsb.tile([1, 512], F32, tag="sm32", bufs=8)
                    nc.vector.tensor_copy(sm32[:], cu[ds(64, 1), :])
                    r32 = rsb.tile([1, 512], F32, tag="r32", bufs=8)
                    nc.vector.reciprocal_approx_fast(r32[:], sm32[:])
                    r16 = rsb.tile([1, 512], BF16, tag="r16", bufs=8)
                    nc.vector.tensor_copy(r16[:], r32[:])
                    cus.append((h, cu, r16, tq))
            pending[tq] = cus

        # tail: chunk 3 epilogue runs exposed; reduce-scatter it as two
        # 256-row halves so the first overlaps the second's out-projection
        last = pending.pop(3)
        for item in last:
            epi_norm(item)
        for tt in range(4):
            epi_outproj(3, tt, dst=ph_c[tt // 2][ds(128 * (tt % 2), 128), :])
            if tt % 2 == 1:
                h = tt // 2
                nc.gpsimd.collective_compute(
                    "ReduceScatter", mybir.AluOpType.add,
                    replica_groups=GROUPS,
                    ins=[ph_c[h].opt()], outs=[rh_c[h].opt()])
                nc.sync.dma_start(out_d[ds(384 + 64 * h, 64), :], rh_c[h][:])

    nc.compile()
    return nc


def _get_nc():
    if "nc" not in _cache:
        _cache["nc"] = _build()
    return _cache["nc"]


def _in_maps(x, Wq, bq, Wk, bk, Wv, bv, Wo, bo):
    bf = ml_dtypes.bfloat16
    maps = []
    for c in range(NCORES):
        b, g = c // 4, c % 4
        j0 = JG * g
        maps.append({
            "xbf": np.ascontiguousarray(x[b]).astype(bf),
            "ident": np.eye(128, dtype=np.float32).astype(bf),
            "wq": np.ascontiguousarray(Wq[:, j0:j0 + JG]).astype(bf),
            "wk": np.ascontiguousarray(Wk[:, j0:j0 + JG]).astype(bf),
            "wv": np.ascontiguousarray(Wv[:, j0:j0 + JG]).astype(bf),
            "wo": np.ascontiguousarray(Wo[j0:j0 + JG, :]).astype(bf),
            "bqc": np.ascontiguousarray(bq[j0:j0 + JG].reshape(2, 128).T),
            "bkc": np.ascontiguousarray(bk[j0:j0 + JG].reshape(2, 128).T),
        })
    return maps


def kernel(x, Wq, bq, Wk, bk, Wv, bv, Wo, bo, _trace=False):
    x, Wq, bq, Wk, bk, Wv, bv, Wo, bo = (
        np.asarray(a, dtype=np.float32)
        for a in (x, Wq, bq, Wk, bk, Wv, bv, Wo, bo))
    nc = _get_nc()
    res = run_bass_kernel_spmd(nc, _in_maps(x, Wq, bq, Wk, bk, Wv, bv, Wo, bo),
                               core_ids=list(range(NCORES)), trace=_trace)
    out = np.empty((B, S, H), np.float32)
    for c in range(NCORES):
        b, g = c // 4, c % 4
        oc = np.asarray(res.results[c]["out"], dtype=np.float32)
        for tq in range(3):
            out[b, 512 * tq + 128 * g:512 * tq + 128 * (g + 1), :] = \
                oc[128 * tq:128 * (tq + 1)]
        for h in range(2):
            r0 = 1536 + 256 * h + 64 * g
            out[b, r0:r0 + 64, :] = oc[384 + 64 * h:384 + 64 * (h + 1)]
    out += bv @ Wo + bo  # exact: softmax rows sum to 1
    if _trace:
        return out, res
    return out


# revision 20
# speedup vs baseline: 1.2393x; 1.0178x over previous
"""Distributed multi-head attention kernel for 8 TRN2 NeuronCores.

Problem: B=2, S=2048, H=1024 (16 heads x 64), fp32 in/out.
Sharding: core c = 4*b + g handles batch b and head-group g (4 heads, 256
hidden cols). Wq/Wk/Wv column-sharded, Wo row-sharded; a per-q-chunk
bf16 ReduceScatter over each 4-core batch group yields each core's
4x128-row slices of the output (overlapped with compute).

v2 dataflow per core (transpose-free attention, bf16 matmuls, fp32 PSUM):
  x^T via DMA-XBAR transpose split across the two HWDGE queues (sync +
  scalar) so descriptor issue is 2x parallel; K/Q projections as N=1024
  matmuls chase the transposes chunk by chunk.
  V projection (x^T-stationary, N=256) is interleaved into chunk 0's
  attention so it hides under the exp/score pipeline.
  scores^T[k,q] = K^T.T@Q^T, two heads packed into PE row groups;
  Pt = exp(scores/8) with no max subtraction (scores ~ N(0,8^2), exact
  softmax); most tiles on ACT, a fraction on DVE via the exp2 bit trick
  (y = round(scores*c1 + c2) as int16 IS the bf16 encoding of
  2^(scores*log2e/8) with piecewise-linear mantissa, ~2% rms weight
  error that cancels in the softmax normalization).
  ctx^T[d,q] (+ sums row via ones column in V) = [V|1].T @ Pt.
  The per-chunk epilogue (normalize via K=1 broadcast matmul of 1/sums,
  out partial = ctx^T-stationary @ Wo, bf16 ReduceScatter) is deferred
  and interleaved into the NEXT chunk's attention so the in-order PE/ACT
  queues never drain between chunks.
bq/bk applied on-device (ACT bias); bv/bo folded in on host (exact:
out += bv@Wo + bo, since softmax rows sum to one).
"""

import sys

for p in ("/opt/trn_rl_repo",):
    if p not in sys.path:
        sys.path.insert(0, p)

import math
from contextlib import ExitStack

import ml_dtypes
import numpy as np

from concourse import bacc, mybir, tile
from concourse.bass import ds
from concourse.bass_utils import run_bass_kernel_spmd

F32 = mybir.dt.float32
BF16 = mybir.dt.bfloat16
I16 = mybir.dt.int16
AF = mybir.ActivationFunctionType

B, S, H = 2, 2048, 1024
NH, D = 16, 64
NCORES = 8
GROUPS = [[0, 1, 2, 3], [4, 5, 6, 7]]
JG = 256           # hidden cols per core (4 heads)
SO = S // 4        # 512 output rows per core after reduce-scatter

# exp2 bit-trick constants: bits_i16 = scores * C1 + C2 viewed as bf16
# encodes 2^(scores/8 * log2 e) with a piecewise-linear mantissa.
EXP_C1 = 128.0 * 0.125 * math.log2(math.e)
EXP_C2 = 128.0 * (127.0 - 0.0579)
# kt indices whose exp tile runs on DVE (per hp pass) in chunks 1..3
DVE_KT = (5, 11)

_cache = {}


def _build():
    nc = bacc.Bacc("TRN2", target_bir_lowering=False, debug=False,
                   num_devices=NCORES)
    x_d = nc.dram_tensor("xbf", [S, H], BF16, kind="ExternalInput")
    id_d = nc.dram_tensor("ident", [128, 128], BF16, kind="ExternalInput")
    wq_d = nc.dram_tensor("wq", [H, JG], BF16, kind="ExternalInput")
    wk_d = nc.dram_tensor("wk", [H, JG], BF16, kind="ExternalInput")
    wv_d = nc.dram_tensor("wv", [H, JG], BF16, kind="ExternalInput")
    wo_d = nc.dram_tensor("wo", [JG, H], BF16, kind="ExternalInput")
    bq_d = nc.dram_tensor("bqc", [128, 2], F32, kind="ExternalInput")
    bk_d = nc.dram_tensor("bkc", [128, 2], F32, kind="ExternalInput")
    out_d = nc.dram_tensor("out", [SO, H], BF16, kind="ExternalOutput")

    def mm(ps, lhsT, rhs, start, stop, tile_position=None):
        nc.tensor.matmul(ps, lhsT, rhs, start=start, stop=stop,
                         tile_position=tile_position)

    with tile.TileContext(nc) as tc, ExitStack() as st:
        consts = st.enter_context(tc.tile_pool(name="consts", bufs=1))
        ones1 = consts.tile([1, 64], BF16)
        nc.vector.memset(ones1[:], 1.0)
        bq_sb = consts.tile([128, 2], F32)
        nc.gpsimd.dma_start(bq_sb[:], bq_d[:, :])
        bk_sb = consts.tile([128, 2], F32)
        nc.gpsimd.dma_start(bk_sb[:], bk_d[:, :])

        wpool = st.enter_context(tc.tile_pool(name="weights", bufs=1))
        w_sb = {}

        def load_w(wname, wd, inner):
            # w*_sb[:, inner*s + j] = W[s*128 + p, j]; on the scalar queue
            wt = wpool.tile([128, 2048], BF16, name=f"{wname}sb",
                            tag=f"{wname}sb")
            nc.scalar.dma_start(
                wt[:].rearrange("p (s j) -> p s j", j=inner),
                wd.ap().rearrange("(s p) j -> p s j", p=128))
            w_sb[wname] = wt

        qkv = st.enter_context(tc.tile_pool(name="qkv", bufs=1))
        qT = [qkv.tile([128, S], BF16, name=f"qT{j}", tag=f"qT{j}")
              for j in range(2)]
        kT = [qkv.tile([128, S], BF16, name=f"kT{j}", tag=f"kT{j}")
              for j in range(2)]
        ctxT = [qkv.tile([128, S], BF16, name=f"cT{j}", tag=f"cT{j}")
                for j in range(2)]
        # V padded per head with a ones column: head h at cols 65h..65h+63
        v_sb = [qkv.tile([128, 260], BF16, name=f"v{i}", tag=f"v{i}")
                for i in range(16)]

        xTp = st.enter_context(tc.tile_pool(name="xT", bufs=1))
        xT = [xTp.tile([128, S], BF16, name=f"xT{s}", tag=f"xT{s}")
              for s in range(8)]
        # x rows staged untransposed for the PE-transpose path (s=2..7)
        xa = [xTp.tile([128, 768], BF16, name=f"xa{t}", tag=f"xa{t}")
              for t in range(16)]

        dram = st.enter_context(tc.tile_pool(name="dram", bufs=1, space="DRAM"))
        partial_c = [dram.tile([512, H], BF16, name=f"pc{i}", tag=f"pc{i}")
                     for i in range(3)]
        rs_c = [dram.tile([128, H], BF16, name=f"rc{i}", tag=f"rc{i}")
                for i in range(3)]
        # last chunk reduce-scatters as two 256-row halves to shrink the tail
        ph_c = [dram.tile([256, H], BF16, name=f"ph{i}", tag=f"ph{i}")
                for i in range(2)]
        rh_c = [dram.tile([64, H], BF16, name=f"rh{i}", tag=f"rh{i}")
                for i in range(2)]

        scps = st.enter_context(tc.tile_pool(name="scps", bufs=2, space="PSUM"))
        ctxps = st.enter_context(tc.tile_pool(name="ctxps", bufs=3,
                                              space="PSUM"))
        psb = st.enter_context(tc.tile_pool(name="psb", bufs=4))
        rsb = st.enter_context(tc.tile_pool(name="rsb", bufs=2))
        osb = st.enter_context(tc.tile_pool(name="osb", bufs=4))

        # ---- Phase A: x^T via XBAR (s=0..2, sync queue only: concurrent
        # XBAR transposes from two HWDGE queues corrupt each other) plus a
        # PE identity-matmul transpose path (s=3..7) fed by plain DMAs ----
        ident = consts.tile([128, 128], BF16)
        nc.scalar.dma_start(ident[:], id_d.ap())
        load_w("wk", wk_d, 256)
        load_w("wq", wq_d, 256)
        for tt in range(16):
            nc.scalar.dma_start(xa[tt][:],
                                x_d.ap()[ds(128 * tt, 128), ds(256, 768)])

        def transposes(tq):
            for s in range(2):
                nc.sync.dma_start(xT[s][:, ds(512 * tq, 512)],
                                  x_d.ap()[ds(512 * tq, 512), ds(128 * s, 128)],
                                  transpose=True)

        def proj_pair(wt, dst, jt, tqp):
            # dst[jt][:, 1024*tqp:+1024] = (W^T x^T) for a 2-chunk slice
            # (two N=512 matmul groups: ISA caps matmul output at 512 elems)
            ps = scps.tile([128, 1024], F32, tag="sps")
            for half in range(2):
                for s in range(8):
                    mm(ps[:, ds(512 * half, 512)],
                       wt[:, ds(256 * s + 128 * jt, 128)],
                       xT[s][:, ds(1024 * tqp + 512 * half, 512)],
                       s == 0, s == 7)
            return ps

        for tq in range(4):
            transposes(tq)
        load_w("wv", wv_d, 256)
        load_w("wo", wo_d, 1024)

        # scoped PSUM bank: PE-transpose ping-pong during phase A, V
        # accumulator during chunk 0, broadcast bank afterwards.
        tp_cm = tc.tile_pool(name="tp", bufs=1, space="PSUM")
        tpp = tp_cm.__enter__()
        tp = tpp.tile([128, 1024], BF16, tag="tp")
        qk_ps = []
        for tq in range(4):
            for s in range(2, 8):
                half = (6 * tq + s - 2) % 2
                for q in range(4):
                    nc.tensor.transpose(
                        tp[:, ds(512 * half + 128 * q, 128)],
                        xa[4 * tq + q][:, ds(128 * (s - 2), 128)], ident[:])
                nc.vector.tensor_copy(xT[s][:, ds(512 * tq, 512)],
                                      tp[:, ds(512 * half, 512)])
            if tq == 1:
                for jt in range(2):
                    qk_ps.append(("k", jt, 0, proj_pair(w_sb["wk"], kT, jt, 0)))
            if tq == 3:
                for jt in range(2):
                    qk_ps.append(("k", jt, 1, proj_pair(w_sb["wk"], kT, jt, 1)))
                for jt in range(2):
                    qk_ps.append(("q", jt, 0, proj_pair(w_sb["wq"], qT, jt, 0)))
        for kind, jt, tqp, ps in qk_ps:
            dst, bias = (kT, bk_sb) if kind == "k" else (qT, bq_sb)
            nc.scalar.activation(dst[jt][:, ds(1024 * tqp, 1024)], ps[:],
                                 AF.Identity, bias=bias[:, ds(jt, 1)])
        del qk_ps

        tp_cm.__exit__(None, None, None)
        vps_cm = tc.tile_pool(name="vps", bufs=1, space="PSUM")
        vps = vps_cm.__enter__()
        bcps = None

        def v_proj(tv):
            ps = vps.tile([128, 256], F32, tag="vps")
            for s in range(8):
                mm(ps[:], xT[s][:, ds(128 * tv, 128)],
                   w_sb["wv"][:, ds(256 * s, 256)], s == 0, s == 7)
            nc.vector.memset(v_sb[tv][:], 1.0)
            nc.vector.tensor_copy(
                v_sb[tv][:].rearrange("p (h c) -> p h c", c=65)[:, :, 0:64],
                ps[:].rearrange("p (h c) -> p h c", c=64))

        def q_proj_half(tq, jt):
            ps = scps.tile([128, 1024], F32, tag="sps")
            for s in range(8):
                mm(ps[:, 0:512], w_sb["wq"][:, ds(256 * s + 128 * jt, 128)],
                   xT[s][:, ds(512 * tq, 512)], s == 0, s == 7)
            nc.scalar.activation(qT[jt][:, ds(512 * tq, 512)], ps[:, 0:512],
                                 AF.Identity, bias=bq_sb[:, ds(jt, 1)])

        # ---- Chunks: attention with deferred, interleaved epilogue ----
        # pending[tq] = list of 4 (h, cu, r16) in normalize order
        pending = {}

        def epi_norm(item):
            # broadcast 1/sums over 64 partitions and scale ctx -> ctxT
            h, cu, r16, tq = item
            hp, po = h // 2, 64 * (h % 2)
            bc = bcps.tile([64, 512], F32, tag="bc")
            mm(bc[:], ones1[:], r16[:], True, True)
            nc.vector.tensor_mul(
                ctxT[hp][ds(po, 64), ds(512 * tq, 512)], cu[0:64, :], bc[:])

        def epi_outproj(tq, tt, dst=None):
            ps = scps.tile([128, 1024], F32, tag="sps")
            for oo in range(2):
                for idx, js in enumerate((1, 0)):
                    mm(ps[:, ds(512 * oo, 512)],
                       ctxT[js][:, ds(512 * tq + 128 * tt, 128)],
                       w_sb["wo"][:, ds(1024 * js + 512 * oo, 512)],
                       idx == 0, idx == 1)
            ot = osb.tile([128, 1024], BF16, tag="ot")
            nc.vector.tensor_copy(ot[:], ps[:])
            if dst is None:
                dst = partial_c[tq][ds(128 * tt, 128), :]
            nc.sync.dma_start(dst, ot[:])

        def epi_rs(tq):
            nc.gpsimd.collective_compute(
                "ReduceScatter", mybir.AluOpType.add,
                replica_groups=GROUPS,
                ins=[partial_c[tq].opt()], outs=[rs_c[tq].opt()])
            nc.sync.dma_start(out_d[ds(128 * tq, 128), :], rs_c[tq][:])

        for tq in range(4):
            prev = pending.pop(tq - 1, None)
            cus = []
            for hp in (1, 0):
                cA = ctxps.tile([65, 512], F32, tag="cps")
                cB = ctxps.tile([65, 512], F32, tag="cps")
                for kt in range(16):
                    # chunk 0: V projection rides along the first hp pass
                    if tq == 0 and hp == 1:
                        v_proj(kt)
                    # deferred epilogue of the previous chunk
                    if prev is not None and hp == 1:
                        if 2 <= kt <= 5:
                            epi_norm(prev[kt - 2])
                        elif kt in (7, 9, 11, 13):
                            epi_outproj(tq - 1, (kt - 7) // 2)
                        elif kt == 15:
                            epi_rs(tq - 1)
                    # Q projection for chunk tq+1 (chunks 1,2 carry Q2,Q3)
                    if tq in (1, 2) and hp == 0 and kt in (2, 6):
                        q_proj_half(tq + 1, kt // 4)
                    sp = scps.tile([128, 1024], F32, tag="sps")
                    mm(sp[:, 0:512],
                       kT[hp][0:64, ds(128 * kt, 128)],
                       qT[hp][0:64, ds(512 * tq, 512)],
                       True, True, tile_position=(0, 0))
                    mm(sp[:, 512:1024],
                       kT[hp][64:128, ds(128 * kt, 128)],
                       qT[hp][64:128, ds(512 * tq, 512)],
                       True, True, tile_position=(64, 0))
                    pt = psb.tile([128, 1024], BF16, tag="pt")
                    if tq > 0 and kt in DVE_KT:
                        nc.vector.tensor_scalar(
                            pt[:].bitcast(I16), sp[:], EXP_C1, EXP_C2,
                            mybir.AluOpType.mult, mybir.AluOpType.add)
                    else:
                        nc.scalar.activation(pt[:], sp[:], AF.Exp, scale=0.125)
                    mm(cA[:], v_sb[kt][:, ds(65 * (2 * hp), 65)],
                       pt[:, 0:512], kt == 0, kt == 15)
                    mm(cB[:], v_sb[kt][:, ds(65 * (2 * hp + 1), 65)],
                       pt[:, 512:1024], kt == 0, kt == 15)
                if tq == 0 and hp == 1:
                    # V accumulator bank becomes the broadcast bank
                    vps_cm.__exit__(None, None, None)
                    bcps_cm = tc.tile_pool(name="bcps", bufs=1, space="PSUM")
                    bcps = bcps_cm.__enter__()
                    st.push(bcps_cm)
                for h, cps in ((2 * hp, cA), (2 * hp + 1, cB)):
                    # evacuate PSUM right away; reciprocal on the thin sums
                    # row, cast bf16 for the K=1 broadcast matmul
                    cu = rsb.tile([65, 512], F32, tag="cu", bufs=8)
                    nc.vector.tensor_copy(cu[:], cps[:])
                    # custom-DVE ops misread nonzero base partitions on HW:
                    # stage the sums row at partition 0 before reciprocal
                    sm32 = rsb.tile([1, 512], F32, tag="sm32", bufs=8)
                    nc.vector.tensor_copy(sm32[:], cu[ds(64, 1), :])
                    r32 = rsb.tile([1, 512], F32, tag="r32", bufs=8)
                    nc.vector.reciprocal_approx_fast(r32[:], sm32[:])
                    r16 = rsb.tile([1, 512], BF16, tag="r16", bufs=8)
                    nc.vector.tensor_copy(r16[:], r32[:])
                    cus.append((h, cu, r16, tq))
            pending[tq] = cus

        # tail: chunk 3 epilogue runs exposed; reduce-scatter it as two
        # 256-row halves so the first overlaps the second's out-projection
        last = pending.pop(3)
        for item in last:
            epi_norm(item)
        for tt in range(4):
            epi_outproj(3, tt, dst=ph_c[tt // 2][ds(128 * (tt % 2), 128), :])
            if tt % 2 == 1:
                h = tt // 2
                nc.gpsimd.collective_compute(
                    "ReduceScatter", mybir.AluOpType.add,
                    replica_groups=GROUPS,
                    ins=[ph_c[h].opt()], outs=[rh_c[h].opt()])
                nc.sync.dma_start(out_d[ds(384 + 64 * h, 64), :], rh_c[h][:])

    nc.compile()
    return nc


def _get_nc():
    if "nc" not in _cache:
        _cache["nc"] = _build()
    return _cache["nc"]


def _in_maps(x, Wq, bq, Wk, bk, Wv, bv, Wo, bo):
    bf = ml_dtypes.bfloat16
    maps = []
    for c in range(NCORES):
        b, g = c // 4, c % 4
        j0 = JG * g
        maps.append({
            "xbf": np.ascontiguousarray(x[b]).astype(bf),
            "ident": np.eye(128, dtype=np.float32).astype(bf),
            "wq": np.ascontiguousarray(Wq[:, j0:j0 + JG]).astype(bf),
            "wk": np.ascontiguousarray(Wk[:, j0:j0 + JG]).astype(bf),
            "wv": np.ascontiguousarray(Wv[:, j0:j0 + JG]).astype(bf),
            "wo": np.ascontiguousarray(Wo[j0:j0 + JG, :]).astype(bf),
            "bqc": np.ascontiguousarray(bq[j0:j0 + JG].reshape(2, 128).T),
            "bkc": np.ascontiguousarray(bk[j0:j0 + JG].reshape(2, 128).T),
        })
    return maps


def kernel(x, Wq, bq, Wk, bk, Wv, bv, Wo, bo, _trace=False):
    x, Wq, bq, Wk, bk, Wv, bv, Wo, bo = (
        np.asarray(a, dtype=np.float32)
        for a in (x, Wq, bq, Wk, bk, Wv, bv, Wo, bo))
    nc = _get_nc()
    res = run_bass_kernel_spmd(nc, _in_maps(x, Wq, bq, Wk, bk, Wv, bv, Wo, bo),
                               core_ids=list(range(NCORES)), trace=_trace)
    out = np.empty((B, S, H), np.float32)
    for c in range(NCORES):
        b, g = c // 4, c % 4
        oc = np.asarray(res.results[c]["out"], dtype=np.float32)
        for tq in range(3):
            out[b, 512 * tq + 128 * g:512 * tq + 128 * (g + 1), :] = \
                oc[128 * tq:128 * (tq + 1)]
        for h in range(2):
            r0 = 1536 + 256 * h + 64 * g
            out[b, r0:r0 + 64, :] = oc[384 + 64 * h:384 + 64 * (h + 1)]
    out += bv @ Wo + bo  # exact: softmax rows sum to 1
    if _trace:
        return out, res
    return out
